# revision 7
# baseline (speedup 1.0000x reference)
"""Local multi-head attention (window=64) Bass/Tile kernel for TRN2.

Sharding: token-parallel. 8 cores = (4 batches) x (2 halves of T=2048).
Each core computes tokens [t0, t0+1024) of one batch row with a 32-token
halo for K/V. No collectives; host concatenates per-core outputs.

Device layouts (per core, all fp32, matmuls run as float32r):
  xT   [1024, 1152]  x^T, halo-padded (col t_loc = global t0-32+t_loc)
  QK^T [2048, 1152]  = W_qk @ x^T   (f-major: Q rows 0:1024, K rows 1024:2048)
  V    [1152, 1024]  = x @ W_v^T    (t-major)
  S^T  [k, q] per (head, 256-query group): 3 chunks of [<=128, 256]
  A^T  [1024, 1024]  normalized attention output, D-major
  y^T  [1024, 1024]  = W_out @ A^T + b_out  (host transposes back)
"""

import os
import numpy as np

try:
    import concourse.bass as bass  # noqa: F401
except ImportError:  # pragma: no cover
    import sys

    sys.path.insert(0, "/opt/trn_rl_repo")

import concourse.bass as bass
import concourse.mybir as mybir
import concourse.tile as tile
from concourse import bacc
from concourse.bass_utils import run_bass_kernel_spmd

B, T, D = 4, 2048, 1024
H, DH = 16, 64
HALO = 32          # window half-width
NCORES = 8
TC = 1024          # tokens per core
TK = TC + 2 * HALO # 1088 keys incl halo
TPAD = 1152        # 9 * 128
NT = TPAD // 128
QG = 256           # query group
NG = TC // QG      # 4
NKC = 3            # key chunks per group: 128+128+64 = 320-key span
F32 = mybir.dt.float32
F32R = mybir.dt.float32r
SCALE = DH ** -0.5

LAST_EXEC_NS = None
_NC = None


def _emit(nc, tc, aps):
    xT, wqkT, wvT, woT, bqk, bvb, bo, maskT, yT = aps
    Exp = mybir.ActivationFunctionType.Exp

    with (
        tc.tile_pool(name="persist", bufs=1) as pp,
        tc.tile_pool(name="consts", bufs=1) as cp,
    ):
        # constants
        ones_f = cp.tile([128, 64], F32, name="ones_f", tag="ones_f")
        nc.vector.memset(ones_f[:], 1.0)
        ones = cp.tile([128, 64], F32R, name="ones", tag="ones")
        nc.vector.tensor_copy(ones[:], ones_f[:])
        bqk_sb = cp.tile([128, 16], F32, name="bqk", tag="bqk")
        nc.sync.dma_start(bqk_sb[:], bqk[:])
        bvb_sb = cp.tile([128, D], F32, name="bvb", tag="bvb")
        nc.sync.dma_start(bvb_sb[:], bvb[:])
        bo_sb = cp.tile([128, 8], F32, name="bo", tag="bo")
        nc.sync.dma_start(bo_sb[:], bo[:])
        msk = []
        for m in range(NG * NKC):
            mt = cp.tile([128, QG], F32, name=f"msk{m}", tag=f"msk{m}")
            nc.sync.dma_start(mt[:], maskT[m])
            msk.append(mt)

        # persistent activations
        qk = [pp.tile([128, TPAD], F32R, name=f"qk{f}", tag=f"qk{f}") for f in range(16)]
        v = [pp.tile([128, D], F32R, name=f"v{t}", tag=f"v{t}") for t in range(NT)]

        with tc.tile_pool(name="xtp", bufs=1) as xp:
            xt = [xp.tile([128, TPAD], F32R, name=f"xt{c}", tag=f"xt{c}") for c in range(8)]
            for c in range(8):
                nc.sync.dma_start(xt[c][:], xT[c * 128:(c + 1) * 128, :])

            # ---- stage B: V = x @ Wv^T + bv  (t-major) ----
            with (
                tc.tile_pool(name="wv", bufs=10) as wvp,
                tc.tile_pool(name="psB", bufs=3, space="PSUM") as psB,
            ):
                for j in range(2):
                    wvt = []
                    for c in range(8):
                        wt = wvp.tile([128, 512], F32R, name="wv", tag="wv")
                        nc.sync.dma_start(
                            wt[:], wvT[c * 128:(c + 1) * 128, j * 512:(j + 1) * 512]
                        )
                        wvt.append(wt)
                    for tb in range(NT):
                        ps = psB.tile([128, 512], F32, name="psB", tag="psB")
                        for c in range(8):
                            nc.tensor.matmul(
                                ps[:],
                                xt[c][:, tb * 128:(tb + 1) * 128],
                                wvt[c][:],
                                start=(c == 0),
                                stop=(c == 7),
                            )
                        nc.vector.tensor_add(
                            v[tb][:, j * 512:(j + 1) * 512],
                            ps[:],
                            bvb_sb[:, j * 512:(j + 1) * 512],
                        )

            # ---- stage A: QK^T = W_qk @ x^T + b  (f-major) ----
            with (
                tc.tile_pool(name="wqk", bufs=12) as wqp,
                tc.tile_pool(name="psA", bufs=3, space="PSUM") as psA,
            ):
                f_order = [f for pair in zip(range(8), range(8, 16)) for f in pair]
                for f in f_order:
                    wts = []
                    for c in range(8):
                        wt = wqp.tile([128, 128], F32R, name="wqk", tag="wqk")
                        nc.sync.dma_start(
                            wt[:], wqkT[c * 128:(c + 1) * 128, f * 128:(f + 1) * 128]
                        )
                        wts.append(wt)
                    for n in range(3):
                        ps = psA.tile([128, 384], F32, name="psA", tag="psA")
                        for c in range(8):
                            nc.tensor.matmul(
                                ps[:],
                                wts[c][:],
                                xt[c][:, n * 384:(n + 1) * 384],
                                start=(c == 0),
                                stop=(c == 7),
                            )
                        nc.vector.tensor_scalar_add(
                            qk[f][:, n * 384:(n + 1) * 384], ps[:], bqk_sb[:, f:f + 1]
                        )

        # ---- stage C: local attention, per (head, 256-query group) ----
        with tc.tile_pool(name="atp", bufs=1) as ap_:
            at = [ap_.tile([128, TC], F32R, name=f"at{i}", tag=f"at{i}") for i in range(8)]
            _attention(nc, tc, qk, v, msk, ones, at)

            # ---- stage D: y^T = W_out @ A^T + b_out ----
            with (
                tc.tile_pool(name="wo", bufs=12) as wop,
                tc.tile_pool(name="yo", bufs=4) as yop,
                tc.tile_pool(name="psD", bufs=3, space="PSUM") as psD,
            ):
                for fo in range(8):
                    wos = []
                    for c in range(8):
                        wt = wop.tile([128, 128], F32R, name="wo", tag="wo")
                        nc.sync.dma_start(
                            wt[:], woT[c * 128:(c + 1) * 128, fo * 128:(fo + 1) * 128]
                        )
                        wos.append(wt)
                    for n in range(2):
                        ps = psD.tile([128, 512], F32, name="psD", tag="psD")
                        for c in range(8):
                            nc.tensor.matmul(
                                ps[:],
                                wos[c][:],
                                at[c][:, n * 512:(n + 1) * 512],
                                start=(c == 0),
                                stop=(c == 7),
                            )
                        yo = yop.tile([128, 512], F32, name="yo", tag="yo")
                        nc.vector.tensor_scalar_add(yo[:], ps[:], bo_sb[:, fo:fo + 1])
                        nc.sync.dma_start(
                            yT[fo * 128:(fo + 1) * 128, n * 512:(n + 1) * 512], yo[:]
                        )


def _attention(nc, tc, qk, v, msk, ones, at):
    Exp = mybir.ActivationFunctionType.Exp

    with (
        tc.tile_pool(name="ep", bufs=8) as ep,
        tc.tile_pool(name="rp", bufs=4) as rp,
        tc.tile_pool(name="psS", bufs=3, space="PSUM") as psS,
        tc.tile_pool(name="psO", bufs=2, space="PSUM") as psO,
        tc.tile_pool(name="psN", bufs=2, space="PSUM") as psN,
    ):
            for h in range(H):
                fq, rq = h // 2, 64 * (h % 2)
                for g in range(NG):
                    qsl = qk[fq][rq:rq + 64, HALO + g * QG: HALO + (g + 1) * QG]
                    es = []
                    for kc in range(NKC):
                        kb = 2 * g + kc
                        kw = 128 if kc < 2 else 64
                        ps_s = psS.tile([128, QG], F32, name="s", tag="s")
                        nc.tensor.matmul(
                            ps_s[:kw, :],
                            qk[8 + fq][rq:rq + 64, kb * 128: kb * 128 + kw],
                            qsl,
                        )
                        e = ep.tile([128, QG], F32R, name="e", tag="e")
                        nc.scalar.activation(e[:kw, :], ps_s[:kw, :], Exp, scale=SCALE)
                        nc.vector.tensor_mul(
                            e[:kw, :], e[:kw, :], msk[NKC * g + kc][:kw, :]
                        )
                        es.append(e)
                    ps_o = psO.tile([64, QG], F32, name="o", tag="o")
                    ps_n = psN.tile([64, QG], F32, name="n", tag="n")
                    for kc in range(NKC):
                        kb = 2 * g + kc
                        kw = 128 if kc < 2 else 64
                        nc.tensor.matmul(
                            ps_o[:],
                            v[kb][:kw, 64 * h: 64 * h + 64],
                            es[kc][:kw, :],
                            start=(kc == 0),
                            stop=(kc == NKC - 1),
                        )
                    for kc in range(NKC):
                        kw = 128 if kc < 2 else 64
                        nc.tensor.matmul(
                            ps_n[:],
                            ones[:kw, :],
                            es[kc][:kw, :],
                            start=(kc == 0),
                            stop=(kc == NKC - 1),
                        )
                    rec = rp.tile([64, QG], F32, name="r", tag="r")
                    nc.vector.reciprocal(rec[:], ps_n[:])
                    nc.vector.tensor_mul(
                        at[h // 2][rq:rq + 64, g * QG:(g + 1) * QG], ps_o[:], rec[:]
                    )


def _build():
    nc = bacc.Bacc(
        "TRN2", target_bir_lowering=False, debug=False, num_devices=NCORES
    )
    xT = nc.dram_tensor("xT", [D, TPAD], F32R, kind="ExternalInput").ap()
    wqkT = nc.dram_tensor("wqkT", [D, 2 * D], F32R, kind="ExternalInput").ap()
    wvT = nc.dram_tensor("wvT", [D, D], F32R, kind="ExternalInput").ap()
    woT = nc.dram_tensor("woT", [D, D], F32R, kind="ExternalInput").ap()
    bqk = nc.dram_tensor("bqk", [128, 16], F32, kind="ExternalInput").ap()
    bvb = nc.dram_tensor("bvb", [128, D], F32, kind="ExternalInput").ap()
    bo = nc.dram_tensor("bo", [128, 8], F32, kind="ExternalInput").ap()
    maskT = nc.dram_tensor("maskT", [NG * NKC, 128, QG], F32, kind="ExternalInput").ap()
    yT = nc.dram_tensor("yT", [D, TC], F32, kind="ExternalOutput").ap()
    with tile.TileContext(nc) as tc:
        _emit(nc, tc, (xT, wqkT, wvT, woT, bqk, bvb, bo, maskT, yT))
    nc.compile()
    return nc


def get_nc():
    global _NC
    if _NC is None:
        _NC = _build()
    return _NC


def _host_inputs(x, W_qkv, b_qkv, W_out, b_out):
    """Shared (weights) + per-core (xT, maskT) host-side prep."""
    wqkT = np.ascontiguousarray(W_qkv[:2 * D].T)
    wvT = np.ascontiguousarray(W_qkv[2 * D:].T)
    woT = np.ascontiguousarray(W_out.T)
    bqk2d = np.ascontiguousarray(b_qkv[:2 * D].reshape(16, 128).T)
    bvb = np.ascontiguousarray(np.broadcast_to(b_qkv[2 * D:], (128, D)))
    bo2d = np.ascontiguousarray(b_out.reshape(8, 128).T)

    in_maps = []
    for core in range(NCORES):
        b, half = divmod(core, 2)
        t0 = TC * half
        xTc = np.zeros((D, TPAD), np.float32)
        lo, hi = max(0, t0 - HALO), min(T, t0 + TC + HALO)
        xTc[:, lo - (t0 - HALO): hi - (t0 - HALO)] = x[b].T[:, lo:hi]

        m = np.zeros((NG * NKC, 128, QG), np.float32)
        for g in range(NG):
            for kc in range(NKC):
                kw = 128 if kc < 2 else 64
                rr = np.arange(128)[:, None]
                cc = np.arange(QG)[None, :]
                k_loc = QG * g + 128 * kc + rr
                q_own = QG * g + cc
                j_glob = t0 - HALO + k_loc
                valid = (
                    (k_loc - q_own >= 0)
                    & (k_loc - q_own <= 2 * HALO)
                    & (j_glob >= 0)
                    & (j_glob < T)
                    & (rr < kw)
                )
                m[NKC * g + kc] = valid.astype(np.float32)
        in_maps.append(
            {
                "xT": xTc,
                "wqkT": wqkT,
                "wvT": wvT,
                "woT": woT,
                "bqk": bqk2d,
                "bvb": bvb,
                "bo": bo2d,
                "maskT": m,
            }
        )
    return in_maps


def _install_profile_shim():
    """Register the NTFF profile hook that this container's antenv lacks."""
    import sys
    import types

    try:
        from antenv import axon_hooks  # noqa: F401
        return
    except ImportError:
        pass
    import antenv
    from trn_agent_boot.trn_boot import _ntff_profile_via_ctypes

    mod = types.ModuleType("antenv.axon_hooks")
    mod._hook = _ntff_profile_via_ctypes("/opt/axon/libaxon_pjrt.so")
    mod.get_axon_ntff_profile_hook = lambda: mod._hook
    mod.set_axon_ntff_profile_hook = lambda h: setattr(mod, "_hook", h)
    sys.modules["antenv.axon_hooks"] = mod
    antenv.axon_hooks = mod

    import concourse.bass_utils as bu

    bu.upload_artifacts = lambda tmpdir: f"local:{tmpdir}"


def kernel(x, W_qkv, b_qkv, W_out, b_out):
    global LAST_EXEC_NS
    if os.environ.get("KERNEL_TRACE"):
        try:
            _install_profile_shim()
        except Exception as e:  # profiling is best-effort
            print(f"profile shim failed: {e}")
    x = np.asarray(x, np.float32)
    W_qkv = np.asarray(W_qkv, np.float32)
    b_qkv = np.asarray(b_qkv, np.float32)
    W_out = np.asarray(W_out, np.float32)
    b_out = np.asarray(b_out, np.float32)

    in_maps = _host_inputs(x, W_qkv, b_qkv, W_out, b_out)
    nc = get_nc()
    res = run_bass_kernel_spmd(
        nc,
        in_maps,
        core_ids=list(range(NCORES)),
        trace=bool(os.environ.get("KERNEL_TRACE")),
    )
    LAST_EXEC_NS = res.exec_time_ns

    out = np.empty((B, T, D), np.float32)
    for core in range(NCORES):
        b, half = divmod(core, 2)
        t0 = TC * half
        out[b, t0:t0 + TC, :] = res.results[core]["yT"].T
    return out


# revision 11
# speedup vs baseline: 1.2811x; 1.2811x over previous
"""Local multi-head attention (window=64) Bass/Tile kernel for TRN2.

Sharding: token-parallel. 8 cores = (4 batches) x (2 halves of T=2048).
Each core computes tokens [t0, t0+1024) of one batch row with a 32-token
halo for K/V. No collectives; host concatenates per-core outputs.

Device layouts (per core):
  xT   [1024, 1152] f32r  x^T, halo-padded (col t_loc = global t0-32+t_loc)
  QK^T [2048, 1152] f32r  = W_qk @ x^T  (f-major: Q rows 0:1024, K 1024:2048)
  V    [1152, 1024] bf16  = x @ W_v^T   (t-major)
  S^T  [k, q] per (head-pair, 256-query group): 3 chunks [<=128, 512]
       (each chunk holds both heads of the pair in column halves)
  A^T  [1024, 1024] f32r  normalized attention out, D-major
  y^T  [1024, 1024] f32   = W_out @ A^T + b_out  (host transposes back)

Scores matmuls contract Dh=64, which runs at half PE rate; the two heads
of a pair are issued as alternating tile_position row-groups (0,0)/(64,0)
into different PSUM banks so they execute concurrently on the array.
The 64-key tail chunk of each group is zero-padded to K=128 for the
AV/rowsum matmuls (K=64 matmuls cost 2x).
"""

import os
import numpy as np
import ml_dtypes

try:
    import concourse.bass as bass  # noqa: F401
except ImportError:  # pragma: no cover
    import sys

    sys.path.insert(0, "/opt/trn_rl_repo")

import concourse.bass as bass
import concourse.mybir as mybir
import concourse.tile as tile
from concourse import bacc
from concourse.bass_utils import run_bass_kernel_spmd

B, T, D = 4, 2048, 1024
H, DH = 16, 64
HALO = 32          # window half-width
NCORES = 8
TC = 1024          # tokens per core
TPAD = 1152        # 9 * 128
NT = TPAD // 128
QG = 256           # query group
NG = TC // QG      # 4
NKC = 3            # key chunks per group: 128+128+64 = 320-key span
F32 = mybir.dt.float32
F32R = mybir.dt.float32r
BF16 = mybir.dt.bfloat16
SCALE = DH ** -0.5

LAST_EXEC_NS = None
_NC = None


def _emit(nc, tc, aps):
    xT, wqkT, wvT, woT, bqk, bvb, bo, maskT, yT = aps
    Exp = mybir.ActivationFunctionType.Exp

    with (
        tc.tile_pool(name="persist", bufs=1) as pp,
        tc.tile_pool(name="consts", bufs=1) as cp,
    ):
        # persistent activations
        qk = [pp.tile([128, TPAD], F32R, name=f"qk{f}", tag=f"qk{f}") for f in range(16)]
        v = [pp.tile([128, D], BF16, name=f"v{t}", tag=f"v{t}") for t in range(NT)]

        with tc.tile_pool(name="xtp", bufs=1) as xp:
            # x^T and first-stage weights first so PE can start ASAP
            xt = [xp.tile([128, TPAD], F32R, name=f"xt{c}", tag=f"xt{c}") for c in range(8)]
            for n in range(3):
                for c in range(8):
                    nc.sync.dma_start(
                        xt[c][:, n * 384:(n + 1) * 384],
                        xT[c * 128:(c + 1) * 128, n * 384:(n + 1) * 384],
                    )

            # constants (after xt so they don't delay the first matmuls)
            ones_f = cp.tile([128, 128], F32, name="ones_f", tag="ones_f")
            nc.vector.memset(ones_f[:], 1.0)
            ones = cp.tile([128, 128], BF16, name="ones", tag="ones")
            nc.vector.tensor_copy(ones[:], ones_f[:])
            bqk_sb = cp.tile([128, 16], F32, name="bqk", tag="bqk")
            nc.sync.dma_start(bqk_sb[:], bqk[:])
            bvb_sb = cp.tile([128, D], F32, name="bvb", tag="bvb")
            nc.sync.dma_start(bvb_sb[:], bvb[:])
            bo_sb = cp.tile([128, 8], F32, name="bo", tag="bo")
            nc.sync.dma_start(bo_sb[:], bo[:])
            msk = []
            for m in range(NG * NKC):
                mt = cp.tile([128, 2 * QG], BF16, name=f"msk{m}", tag=f"msk{m}")
                nc.sync.dma_start(mt[:], maskT[m])
                msk.append(mt)

            # ---- stage B: V = x @ Wv^T + bv  (t-major, bf16) ----
            with (
                tc.tile_pool(name="wv", bufs=10) as wvp,
                tc.tile_pool(name="psB", bufs=3, space="PSUM") as psB,
            ):
                for j in range(2):
                    wvt = []
                    for c in range(8):
                        wt = wvp.tile([128, 512], F32R, name="wv", tag="wv")
                        nc.sync.dma_start(
                            wt[:], wvT[c * 128:(c + 1) * 128, j * 512:(j + 1) * 512]
                        )
                        wvt.append(wt)
                    for tb in range(NT):
                        ps = psB.tile([128, 512], F32, name="psB", tag="psB")
                        for c in range(8):
                            nc.tensor.matmul(
                                ps[:],
                                xt[c][:, tb * 128:(tb + 1) * 128],
                                wvt[c][:],
                                start=(c == 0),
                                stop=(c == 7),
                            )
                        nc.vector.tensor_add(
                            v[tb][:, j * 512:(j + 1) * 512],
                            ps[:],
                            bvb_sb[:, j * 512:(j + 1) * 512],
                        )

            # ---- stage A: QK^T = W_qk @ x^T + b  (f-major) ----
            with (
                tc.tile_pool(name="wqk", bufs=12) as wqp,
                tc.tile_pool(name="psA", bufs=3, space="PSUM") as psA,
            ):
                f_order = [f for pair in zip(range(8), range(8, 16)) for f in pair]
                for f in f_order:
                    wts = []
                    for c in range(8):
                        wt = wqp.tile([128, 128], F32R, name="wqk", tag="wqk")
                        nc.sync.dma_start(
                            wt[:], wqkT[c * 128:(c + 1) * 128, f * 128:(f + 1) * 128]
                        )
                        wts.append(wt)
                    for n in range(3):
                        ps = psA.tile([128, 384], F32, name="psA", tag="psA")
                        for c in range(8):
                            nc.tensor.matmul(
                                ps[:],
                                wts[c][:],
                                xt[c][:, n * 384:(n + 1) * 384],
                                start=(c == 0),
                                stop=(c == 7),
                            )
                        nc.vector.tensor_scalar_add(
                            qk[f][:, n * 384:(n + 1) * 384], ps[:], bqk_sb[:, f:f + 1]
                        )

        # ---- stages C+D ----
        with tc.tile_pool(name="atp", bufs=1) as ap_:
            at = [ap_.tile([128, TC], F32R, name=f"at{i}", tag=f"at{i}") for i in range(8)]
            _attention(nc, tc, qk, v, msk, ones, at)

            # ---- stage D: y^T = W_out @ A^T + b_out ----
            with (
                tc.tile_pool(name="wo", bufs=24) as wop,
                tc.tile_pool(name="yo", bufs=4) as yop,
                tc.tile_pool(name="psD", bufs=3, space="PSUM") as psD,
            ):
                for fo in range(8):
                    wos = []
                    for c in range(8):
                        wt = wop.tile([128, 128], F32R, name="wo", tag="wo")
                        nc.sync.dma_start(
                            wt[:], woT[c * 128:(c + 1) * 128, fo * 128:(fo + 1) * 128]
                        )
                        wos.append(wt)
                    for n in range(2):
                        ps = psD.tile([128, 512], F32, name="psD", tag="psD")
                        for c in range(8):
                            nc.tensor.matmul(
                                ps[:],
                                wos[c][:],
                                at[c][:, n * 512:(n + 1) * 512],
                                start=(c == 0),
                                stop=(c == 7),
                            )
                        yo = yop.tile([128, 512], F32, name="yo", tag="yo")
                        nc.vector.tensor_scalar_add(yo[:], ps[:], bo_sb[:, fo:fo + 1])
                        nc.sync.dma_start(
                            yT[fo * 128:(fo + 1) * 128, n * 512:(n + 1) * 512], yo[:]
                        )


def _attention(nc, tc, qk, v, msk, ones, at):
    Exp = mybir.ActivationFunctionType.Exp

    with (
        tc.tile_pool(name="ep", bufs=6) as ep,
        tc.tile_pool(name="rp", bufs=3) as rp,
        tc.tile_pool(name="psS", bufs=4, space="PSUM") as psS,
        tc.tile_pool(name="psO", bufs=2, space="PSUM") as psO,
        tc.tile_pool(name="psN", bufs=2, space="PSUM") as psN,
    ):
        for p in range(H // 2):          # head pair (2p, 2p+1); qk f-tile = p
            for g in range(NG):
                # --- scores S^T + P = exp(S/8)*mask, per key chunk.
                # Contraction is Dh=64 (half PE rate); the two heads use
                # different row ranges (base_partition 0 / 64) and separate
                # PSUM tiles, so the array row-groups overlap them. PE
                # matmuls must write a tile's full free range; ACT then
                # merges the pair into column halves of one [128, 512] e
                # tile so the mask-mul and AV/rowsum rhs stay big. ---
                es = []
                for kc in range(NKC):
                    kb = 2 * g + kc
                    kw = 128 if kc < 2 else 64
                    sh = [
                        psS.tile([128, QG], F32, name="s", tag="s", bufs=4)
                        for _ in range(2)
                    ]
                    for hh in range(2):
                        rr = slice(64 * hh, 64 * hh + 64)
                        nc.tensor.matmul(
                            sh[hh][:kw, :],
                            qk[8 + p][rr, kb * 128: kb * 128 + kw],
                            qk[p][rr, HALO + g * QG: HALO + (g + 1) * QG],
                        )
                    if kc < 2:
                        e = ep.tile([128, 2 * QG], BF16, name="e", tag="e", bufs=4)
                    else:
                        e = ep.tile([128, 2 * QG], BF16, name="e2", tag="e2", bufs=2)
                        nc.vector.memset(e[64:128, :], 0.0)
                    for hh in range(2):
                        nc.scalar.activation(
                            e[:kw, QG * hh: QG * (hh + 1)],
                            sh[hh][:kw, :],
                            Exp,
                            scale=SCALE,
                        )
                    mul = nc.gpsimd.tensor_mul if kc == 0 else nc.vector.tensor_mul
                    mul(e[:kw, :], e[:kw, :], msk[NKC * g + kc][:kw, :])
                    es.append(e)
                # --- O^T quadrants + row-sums, both heads per matmul ---
                ps_o = psO.tile([128, 2 * QG], F32, name="o", tag="o")
                ps_n = psN.tile([128, 2 * QG], F32, name="n", tag="n")
                for kc in range(NKC):
                    kb = 2 * g + kc
                    nc.tensor.matmul(
                        ps_o[:],
                        v[kb][:, 128 * p: 128 * (p + 1)],
                        es[kc][:, :],
                        start=(kc == 0),
                        stop=(kc == NKC - 1),
                    )
                for kc in range(NKC):
                    nc.tensor.matmul(
                        ps_n[:],
                        ones[:, :],
                        es[kc][:, :],
                        start=(kc == 0),
                        stop=(kc == NKC - 1),
                    )
                rec = rp.tile([128, 2 * QG], F32, name="r", tag="r")
                nc.vector.reciprocal_approx_fast(rec[:], ps_n[:])
                gc = slice(g * QG, (g + 1) * QG)
                nc.vector.tensor_mul(
                    at[p][0:64, gc], ps_o[0:64, 0:QG], rec[0:64, 0:QG]
                )
                nc.vector.tensor_mul(
                    at[p][64:128, gc], ps_o[64:128, QG:2 * QG], rec[64:128, QG:2 * QG]
                )


def _build():
    nc = bacc.Bacc(
        "TRN2", target_bir_lowering=False, debug=False, num_devices=NCORES
    )
    xT = nc.dram_tensor("xT", [D, TPAD], F32R, kind="ExternalInput").ap()
    wqkT = nc.dram_tensor("wqkT", [D, 2 * D], F32R, kind="ExternalInput").ap()
    wvT = nc.dram_tensor("wvT", [D, D], F32R, kind="ExternalInput").ap()
    woT = nc.dram_tensor("woT", [D, D], F32R, kind="ExternalInput").ap()
    bqk = nc.dram_tensor("bqk", [128, 16], F32, kind="ExternalInput").ap()
    bvb = nc.dram_tensor("bvb", [128, D], F32, kind="ExternalInput").ap()
    bo = nc.dram_tensor("bo", [128, 8], F32, kind="ExternalInput").ap()
    maskT = nc.dram_tensor(
        "maskT", [NG * NKC, 128, 2 * QG], BF16, kind="ExternalInput"
    ).ap()
    yT = nc.dram_tensor("yT", [D, TC], F32, kind="ExternalOutput").ap()
    with tile.TileContext(nc) as tc:
        _emit(nc, tc, (xT, wqkT, wvT, woT, bqk, bvb, bo, maskT, yT))
    nc.compile()
    return nc


def get_nc():
    global _NC
    if _NC is None:
        _NC = _build()
    return _NC


def _host_inputs(x, W_qkv, b_qkv, W_out, b_out):
    """Shared (weights) + per-core (xT, maskT) host-side prep."""
    wqkT = np.ascontiguousarray(W_qkv[:2 * D].T)
    wvT = np.ascontiguousarray(W_qkv[2 * D:].T)
    woT = np.ascontiguousarray(W_out.T)
    bqk2d = np.ascontiguousarray(b_qkv[:2 * D].reshape(16, 128).T)
    bvb = np.ascontiguousarray(np.broadcast_to(b_qkv[2 * D:], (128, D)))
    bo2d = np.ascontiguousarray(b_out.reshape(8, 128).T)

    in_maps = []
    for core in range(NCORES):
        b, half = divmod(core, 2)
        t0 = TC * half
        xTc = np.zeros((D, TPAD), np.float32)
        lo, hi = max(0, t0 - HALO), min(T, t0 + TC + HALO)
        xTc[:, lo - (t0 - HALO): hi - (t0 - HALO)] = x[b].T[:, lo:hi]

        m = np.zeros((NG * NKC, 128, QG), np.float32)
        for g in range(NG):
            for kc in range(NKC):
                kw = 128 if kc < 2 else 64
                rr = np.arange(128)[:, None]
                cc = np.arange(QG)[None, :]
                k_loc = QG * g + 128 * kc + rr
                q_own = QG * g + cc
                j_glob = t0 - HALO + k_loc
                valid = (
                    (k_loc - q_own >= 0)
                    & (k_loc - q_own <= 2 * HALO)
                    & (j_glob >= 0)
                    & (j_glob < T)
                    & (rr < kw)
                )
                m[NKC * g + kc] = valid.astype(np.float32)
        # doubled along columns: the two heads of a pair share the mask
        md = np.concatenate([m, m], axis=2).astype(ml_dtypes.bfloat16)
        in_maps.append(
            {
                "xT": xTc,
                "wqkT": wqkT,
                "wvT": wvT,
                "woT": woT,
                "bqk": bqk2d,
                "bvb": bvb,
                "bo": bo2d,
                "maskT": md,
            }
        )
    return in_maps


def _install_profile_shim():
    """Register the NTFF profile hook that this container's antenv lacks."""
    import sys
    import types

    try:
        from antenv import axon_hooks  # noqa: F401
        return
    except ImportError:
        pass
    import antenv
    from trn_agent_boot.trn_boot import _ntff_profile_via_ctypes

    mod = types.ModuleType("antenv.axon_hooks")
    mod._hook = _ntff_profile_via_ctypes("/opt/axon/libaxon_pjrt.so")
    mod.get_axon_ntff_profile_hook = lambda: mod._hook
    mod.set_axon_ntff_profile_hook = lambda h: setattr(mod, "_hook", h)
    sys.modules["antenv.axon_hooks"] = mod
    antenv.axon_hooks = mod

    import concourse.bass_utils as bu

    bu.upload_artifacts = lambda tmpdir: f"local:{tmpdir}"


def kernel(x, W_qkv, b_qkv, W_out, b_out):
    global LAST_EXEC_NS
    if os.environ.get("KERNEL_TRACE"):
        try:
            _install_profile_shim()
        except Exception as e:  # profiling is best-effort
            print(f"profile shim failed: {e}")
    x = np.asarray(x, np.float32)
    W_qkv = np.asarray(W_qkv, np.float32)
    b_qkv = np.asarray(b_qkv, np.float32)
    W_out = np.asarray(W_out, np.float32)
    b_out = np.asarray(b_out, np.float32)

    in_maps = _host_inputs(x, W_qkv, b_qkv, W_out, b_out)
    nc = get_nc()
    res = run_bass_kernel_spmd(
        nc,
        in_maps,
        core_ids=list(range(NCORES)),
        trace=bool(os.environ.get("KERNEL_TRACE")),
    )
    LAST_EXEC_NS = res.exec_time_ns

    out = np.empty((B, T, D), np.float32)
    for core in range(NCORES):
        b, half = divmod(core, 2)
        t0 = TC * half
        out[b, t0:t0 + TC, :] = res.results[core]["yT"].T
    return out
